# revision 7
# baseline (speedup 1.0000x reference)
"""Complex-valued multi-head attention (B=4, S=1024, D=128, H=8) on 8 TRN2 cores.

Sharding: tensor-parallel over heads -- one head per NeuronCore. Each core
computes its head's complex Q/K projections, a fused V*Wo projection,
complex-magnitude-softmax attention, and its partial contribution to the
output; the host sums the 8 partial outputs (the W_o contraction over heads).

Key structural choices (driven by the TRN2 instruction-cost model):
  - W_o folded into W_v on the host: Wc_h = Wo_h @ Wv_h (complex), so
    attn @ (x_v @ Wc^T) IS the per-head output partial -- no O transpose,
    no separate W_o matmuls on the PE.
  - Q^T, K^T computed as [d, {r,i}, s] via lhsT=W^T, rhs=x^T; -K_i^T by a
    Pool multiply with a -1 constant (TensorScalar is not a Pool opcode).
  - Scores computed transposed S^T[k, q] so E = exp(|s|/sqrt(D)) lands in
    the layout attn@V needs as lhsT.
  - Softmax chain balanced so every engine stays under the PE's ~26us/batch
    (all ops verified against the hardware ISA):
      ACT:  strip = Square(psr)        (PSUM->SBUF; square is in every
                                        act table)
      DVE:  c = copy(psi) as bf16      (PSUM->SBUF downcast)
      DVE:  u = c*c (bf16)             (all-2-byte SBUF op -> 2x DVE rate)
      Pool: strip += u                 (in-place add, Pool is cheap here)
      ACT:  strip = Sqrt(strip/D)      (batched per batch)
      ACT:  strip = Exp(strip)         (scale folded into Sqrt's input)
  - x loads and output stores are triggered from the SP engine; attn output
    normalization 1/Z via per-partition scale on DVE.
"""

import numpy as np

import concourse.bacc as bacc
import concourse.mybir as mybir
import concourse.tile as tile
from concourse.bass_utils import run_bass_kernel_spmd

B, S, D, H = 4, 1024, 128, 8
BS = B * S
P = 128
F32 = mybir.dt.float32
F32R = mybir.dt.float32r
BF16 = mybir.dt.bfloat16

X_NAMES = ("xqr", "xqi", "xkr", "xki", "xvr", "xvi")
W1_NAMES = ("wqr", "wqi", "nwqi", "wkr", "wki", "nwki")
W2_NAMES = ("vc1", "vc2")


W1_OFF = {nm: i * P for i, nm in enumerate(W1_NAMES)}
W2_OFF = {nm: 6 * P + i * 2 * P for i, nm in enumerate(W2_NAMES)}
ONES_OFF = 6 * P + 2 * 2 * P        # 1280
WPACK_COLS = ONES_OFF + 16          # 1296


def build_nc():
    nc = bacc.Bacc()
    xall = nc.dram_tensor("xall", [P, 8 * 6 * 512], F32R, kind="ExternalInput")
    wpack = nc.dram_tensor("wpack", [P, WPACK_COLS], F32R, kind="ExternalInput")
    y = nc.dram_tensor("y", [BS, 2 * P], F32, kind="ExternalOutput")
    xv = xall.rearrange("p (t n c) -> p t n c", t=8, n=6)

    AF = mybir.ActivationFunctionType

    with tile.TileContext(nc) as tc:
        with (
            tc.tile_pool(name="consts", bufs=1) as consts,
            tc.tile_pool(name="xp", bufs=3) as xp,
            tc.tile_pool(name="qk", bufs=2) as qk,
            tc.tile_pool(name="vp", bufs=2) as vp,
            tc.tile_pool(name="sp", bufs=2) as sp,
            tc.tile_pool(name="cp", bufs=4) as cpool,
            tc.tile_pool(name="up", bufs=4) as upool,
            tc.tile_pool(name="yp", bufs=2) as yp,
            tc.tile_pool(name="rp", bufs=8) as rp,
            tc.tile_pool(name="ps", bufs=3, space="PSUM") as ps,
            tc.tile_pool(name="po", bufs=2, space="PSUM") as po,
        ):
            wp = consts.tile([P, WPACK_COLS], F32R, name="wp")
            # split the first weight load so Q-projection weights land first
            nc.sync.dma_start(wp[:, 0:384], wpack[:, 0:384])
            xt00 = xp.tile([P, 6, 512], F32R, name="xt", tag="xt")
            for dd in range(3):
                nc.sync.dma_start(
                    xt00[:, 2 * dd : 2 * dd + 2, :], xv[:, 0, 2 * dd : 2 * dd + 2, :]
                )
            nc.sync.dma_start(wp[:, 384:768], wpack[:, 384:768])
            nc.sync.dma_start(wp[:, 768:WPACK_COLS], wpack[:, 768:WPACK_COLS])
            wt = {nm: wp[:, off : off + P] for nm, off in W1_OFF.items()}
            wt.update({nm: wp[:, off : off + 2 * P] for nm, off in W2_OFF.items()})
            ones16 = wp[:, ONES_OFF : ONES_OFF + 16]
            negones = consts.tile([P, 512], F32, name="negones")
            nc.gpsimd.memset(negones, -1.0)

            qc_all, kc_all, nki_all, v_all = {}, {}, {}, {}

            def proj(b, pre=None):
                # qcat/kcat: [d, {r,i}, s] per-batch projection outputs
                qcat = qk.tile([P, 2, S], F32R, name="qcat", tag="qcat")
                kcat = qk.tile([P, 2, S], F32R, name="kcat", tag="kcat")
                nkiT = qk.tile([P, S], F32R, name="nkiT", tag="nkiT")
                vcat = vp.tile([P, 8, 258], F32R, name="vcat", tag="vcat")
                nc.sync.dma_start(
                    vcat[:, :, 256:258],
                    ones16.rearrange("p (a c) -> p a c", c=2),
                )
                for t2 in range(2):
                    cols = slice(t2 * 512, (t2 + 1) * 512)
                    if pre is not None and t2 in pre:
                        xt = pre[t2]
                    else:
                        xt = xp.tile([P, 6, 512], F32R, name="xt", tag="xt")
                        for dd in range(3):
                            nc.sync.dma_start(
                                xt[:, 2 * dd : 2 * dd + 2, :],
                                xv[:, b * 2 + t2, 2 * dd : 2 * dd + 2, :],
                            )
                    xqr, xqi = xt[:, 0, :], xt[:, 1, :]
                    xkr, xki = xt[:, 2, :], xt[:, 3, :]
                    xvr, xvi = xt[:, 4, :], xt[:, 5, :]

                    pqr = ps.tile([P, 512], F32, name="pqr", tag="ma")
                    nc.tensor.matmul(pqr, wt["wqr"], xqr, start=True, stop=False)
                    nc.tensor.matmul(pqr, wt["nwqi"], xqi, start=False, stop=True)
                    nc.vector.tensor_copy(qcat[:, 0, cols], pqr)
                    pqi = ps.tile([P, 512], F32, name="pqi", tag="mb")
                    nc.tensor.matmul(pqi, wt["wqi"], xqr, start=True, stop=False)
                    nc.tensor.matmul(pqi, wt["wqr"], xqi, start=False, stop=True)
                    nc.vector.tensor_copy(qcat[:, 1, cols], pqi)

                    pkr = ps.tile([P, 512], F32, name="pkr", tag="ma")
                    nc.tensor.matmul(pkr, wt["wkr"], xkr, start=True, stop=False)
                    nc.tensor.matmul(pkr, wt["nwki"], xki, start=False, stop=True)
                    nc.vector.tensor_copy(kcat[:, 0, cols], pkr)
                    pki = ps.tile([P, 512], F32, name="pki", tag="mb")
                    nc.tensor.matmul(pki, wt["wki"], xkr, start=True, stop=False)
                    nc.tensor.matmul(pki, wt["wkr"], xki, start=False, stop=True)
                    nc.vector.tensor_copy(kcat[:, 1, cols], pki)

                    nc.gpsimd.tensor_mul(nkiT[:, cols], kcat[:, 1, cols], negones)

                    for c2 in range(2):
                        gc = t2 * 4 + c2 * 2
                        pv = ps.tile([P, 512], F32, name="pv", tag="ma" if c2 == 0 else "mb")
                        for j in range(2):
                            cc = slice((c2 * 2 + j) * 128, (c2 * 2 + j + 1) * 128)
                            h = slice(j * 256, (j + 1) * 256)
                            nc.tensor.matmul(pv[:, h], xvr[:, cc], wt["vc1"], start=True, stop=False)
                            nc.tensor.matmul(pv[:, h], xvi[:, cc], wt["vc2"], start=False, stop=True)
                        nc.vector.tensor_copy(
                            vcat[:, gc : gc + 2, 0:256],
                            pv.rearrange("p (a c) -> p a c", a=2),
                        )
                qc_all[b] = qcat
                kc_all[b] = kcat
                nki_all[b] = nkiT
                v_all[b] = vcat

            def scores(b, qt, strip):
                qcat, kcat, nkiT = qc_all[b], kc_all[b], nki_all[b]
                qcols = slice(qt * 512, (qt + 1) * 512)
                for kc in range(8):
                    sl8 = qt * 8 + kc
                    kcols = slice(kc * 128, (kc + 1) * 128)
                    psr = ps.tile([P, 512], F32, name="psr", tag="ma")
                    nc.tensor.matmul(psr, kcat[:, 0, kcols], qcat[:, 0, qcols], start=True, stop=False)
                    nc.tensor.matmul(psr, nkiT[:, kcols], qcat[:, 1, qcols], start=False, stop=True)
                    psi = ps.tile([P, 512], F32, name="psi", tag="mb")
                    nc.tensor.matmul(psi, kcat[:, 1, kcols], qcat[:, 0, qcols], start=True, stop=False)
                    nc.tensor.matmul(psi, kcat[:, 0, kcols], qcat[:, 1, qcols], start=False, stop=True)

                    # |s|^2: ACT squares sr into the strip (7 of 8 strips,
                    # DVE-chain for the last); DVE copies si out as bf16 and
                    # squares at the 2-byte 2x rate; Pool adds.
                    if kc < 7:
                        nc.scalar.square(strip[:, sl8, :], psr)
                    else:
                        cr = cpool.tile([P, 512], BF16, name="cr", tag="c")
                        nc.vector.tensor_copy(cr, psr)
                        nc.vector.tensor_mul(strip[:, sl8, :], cr, cr)
                    c = cpool.tile([P, 512], BF16, name="c", tag="c")
                    nc.vector.tensor_copy(c, psi)
                    u = upool.tile([P, 512], BF16, name="u", tag="u")
                    nc.vector.tensor_mul(u, c, c)
                    nc.gpsimd.tensor_add(strip[:, sl8, :], strip[:, sl8, :], u)

            def softmax(b, strip, qts, chunks=1):
                # as few Sqrt/Exp instructions as possible: the scheduler
                # cannot interleave within one instruction, so table loads
                # stay at 2 per batch
                n = (8 * len(qts)) // chunks
                lo = qts[0] * 8
                for ch in range(chunks):
                    hs = strip[:, lo + ch * n : lo + (ch + 1) * n, :]
                    nc.scalar.activation(hs, hs, AF.Sqrt, scale=1.0 / D)
                for ch in range(chunks):
                    hs = strip[:, lo + ch * n : lo + (ch + 1) * n, :]
                    nc.scalar.activation(hs, hs, AF.Exp)

            def attn(b, qt, strip):
                vcat = v_all[b]
                ybuf = yp.tile([P, 4, 256], F32, name="ybuf", tag="ybuf")
                base = b * S + qt * 512
                for qc in range(4):
                    qsub = slice(qc * 128, (qc + 1) * 128)
                    pso = po.tile([P, 258], F32, name="pso", tag="o")
                    for kc in range(8):
                        nc.tensor.matmul(
                            pso, strip[:, qt * 8 + kc, qsub], vcat[:, kc, :],
                            start=(kc == 0), stop=(kc == 7),
                        )
                    rec = rp.tile([P, 1], F32, name="rec", tag="rec")
                    nc.vector.reciprocal(rec, pso[:, 256:257])
                    nc.vector.tensor_scalar_mul(ybuf[:, qc, :], pso[:, 0:256], rec)
                    if qc % 2 == 1:
                        nc.sync.dma_start(
                            y[base + (qc - 1) * 128 : base + (qc + 1) * 128, :]
                            .rearrange("(a p) c -> p a c", p=P),
                            ybuf[:, qc - 1 : qc + 1, :],
                        )

            pend = []
            for b in range(B):
                proj(b, pre={0: xt00} if b == 0 else None)
                strip = sp.tile([P, 16, 512], F32R, name="strip", tag="strip")
                if b < B - 1:
                    scores(b, 0, strip)
                    if len(pend) > 2:
                        attn(*pend.pop(0))
                    scores(b, 1, strip)
                    if len(pend) > 1:
                        attn(*pend.pop(0))
                    softmax(b, strip, (0, 1))
                    pend += [(b, 0, strip), (b, 1, strip)]
                else:
                    # last batch: per-qt halves so attn can start sooner;
                    # final half chunked so the tail drains early
                    for qt in range(2):
                        scores(b, qt, strip)
                        if pend:
                            attn(*pend.pop(0))
                        softmax(b, strip, (qt,), chunks=2 if qt == 1 else 1)
                        pend.append((b, qt, strip))
            for item in pend:
                attn(*item)
    nc.finalize()
    return nc


_NC = None


def _get_nc():
    global _NC
    if _NC is None:
        _NC = build_nc()
    return _NC


def make_in_maps(inputs):
    """Shard full inputs into 8 per-core input maps (head h -> core h)."""
    f = np.float32
    xT = {}
    for src_nm, nm in (("q_r", "xqr"), ("q_i", "xqi"), ("k_r", "xkr"),
                       ("k_i", "xki"), ("v_r", "xvr"), ("v_i", "xvi")):
        xT[nm] = np.asarray(inputs[src_nm], f).reshape(BS, D).T
    # xall layout: [P, t(8), nm(6), 512]
    stack = np.stack([xT[nm].reshape(P, 8, 512) for nm in X_NAMES], axis=2)
    xall = np.ascontiguousarray(stack.reshape(P, 8 * 6 * 512))

    Wq_r = np.asarray(inputs["Wq_r"], f)
    Wq_i = np.asarray(inputs["Wq_i"], f)
    Wk_r = np.asarray(inputs["Wk_r"], f)
    Wk_i = np.asarray(inputs["Wk_i"], f)
    Wv_r = np.asarray(inputs["Wv_r"], f)
    Wv_i = np.asarray(inputs["Wv_i"], f)
    Wo_r = np.asarray(inputs["Wo_r"], f)
    Wo_i = np.asarray(inputs["Wo_i"], f)

    in_maps = []
    for h in range(H):
        sl = slice(h * D, (h + 1) * D)
        # Fold W_o into W_v: Wc = Wo_h @ Wv_h (complex), so that
        # x_v @ Wc^T = (x_v @ Wv_h^T) @ Wo_h^T = V @ Wo_h^T.
        Wor, Woi = Wo_r[:, sl], Wo_i[:, sl]
        Wvr, Wvi = Wv_r[sl], Wv_i[sl]
        Wc_r = (Wor @ Wvr - Woi @ Wvi).astype(f)
        Wc_i = (Wor @ Wvi + Woi @ Wvr).astype(f)
        w = {
            "wqr": Wq_r[sl].T, "wqi": Wq_i[sl].T, "nwqi": -Wq_i[sl].T,
            "wkr": Wk_r[sl].T, "wki": Wk_i[sl].T, "nwki": -Wk_i[sl].T,
            "vc1": np.concatenate([Wc_r.T, Wc_i.T], axis=1),
            "vc2": np.concatenate([-Wc_i.T, Wc_r.T], axis=1),
        }
        wpack = np.zeros((P, WPACK_COLS), f)
        for nm, off in W1_OFF.items():
            wpack[:, off : off + P] = w[nm]
        for nm, off in W2_OFF.items():
            wpack[:, off : off + 2 * P] = w[nm]
        wpack[:, ONES_OFF : ONES_OFF + 16] = 1.0
        in_maps.append({"xall": xall, "wpack": wpack})
    return in_maps


def run(inputs, trace=False):
    nc = _get_nc()
    in_maps = make_in_maps(inputs)
    res = run_bass_kernel_spmd(nc, in_maps, core_ids=list(range(H)), trace=trace)
    ysum = np.zeros((BS, 2 * P), np.float64)
    for r in res.results:
        ysum += r["y"].astype(np.float64)
    yr = ysum[:, :P].reshape(B, S, D)
    yi = ysum[:, P:].reshape(B, S, D)
    out = (yr + 1j * yi).astype(np.complex64)
    return out, res


def kernel(**inputs):
    out, _ = run(inputs, trace=False)
    return out
